# revision 1
# baseline (speedup 1.0000x reference)
"""CKConv (continuous-kernel causal conv) Trainium2 Bass kernel.

Problem: out[b,o,t] = sum_{ci,k<=t} g[o,ci,k] * x[b,ci,t-k] + bias[o]
with g generated by a tiny SIREN net on relative positions.
Shapes: B=4, CIN=32, COUT=64, T=2048, kernel length K=T+1 (tap 2048 never
contributes for t < T, so only taps 0..2047 are computed).

Sharding: 8 cores = (batch b in 0..3) x (input-channel half h in 0..1).
Each core computes a partial over its 16 input channels for all 64 output
channels; the host adds the two halves and the bias (exact fp32 adds).

Formulation (x-stationary): time tiles of 128. For output tile tt and tap
tile j, the contribution is Xwin(d=tt-j).T @ G(j) where Xwin(d)[r, tloc] =
xpad(128d + tloc + r - 127) is a 128x128 window of the shifted-replicated
input (im2col by a single overlapping-window DMA, partition step +1), and
G(j)[r, o] = g[o, cl, 128j + 127 - r]. The within-tile tap reversal is
obtained for free by feeding the SIREN a block-reversed position vector.
PSUM tile w in {0,1} holds t in [1024w, 1024w+1024) as (tloc, (beta, o));
one matmul per (cl, w, d) covers all valid beta blocks at once (moving
operand with 2 free dims), accumulating over cl and d in PSUM.

Matmul dtype bfloat16 (default): ~3e-3 max-rel / ~3e-4 rms-rel error.\nfloat32r alternative (~2.5e-4 max-rel) runs ~13% slower (4x penalty at N<256).
"""

import numpy as np

B, CIN, COUT, T = 4, 32, 64, 2048
DK = 16
N_CORES = 8
CPC = CIN // 2          # channels per core = 16
XPAD_W = 2560           # 512 left zeros + 2048 data
XC_W = 2432             # im2col window columns
GT_COLS = 16 * 1024     # (jt, cl, o) -> jt*1024 + cl*64 + o


def _build_program(om2: float, dt_conv_name: str):
    import concourse.bass as bass
    import concourse.mybir as mybir
    import concourse.tile as tile
    from concourse import bacc
    from concourse.masks import make_identity

    F32 = mybir.dt.float32
    DTC = getattr(mybir.dt, dt_conv_name)
    AF = mybir.ActivationFunctionType

    nc = bacc.Bacc("TRN2", target_bir_lowering=False, debug=False,
                   num_devices=N_CORES)

    xs = nc.dram_tensor("xs", [CPC, T], F32, kind="ExternalInput")
    posr = nc.dram_tensor("posr", [DK, T], F32, kind="ExternalInput")
    w1v = nc.dram_tensor("w1v", [DK, 1], F32, kind="ExternalInput")
    b1v = nc.dram_tensor("b1v", [DK, 1], F32, kind="ExternalInput")
    w2t = nc.dram_tensor("w2t", [DK, DK], F32, kind="ExternalInput")
    b2v = nc.dram_tensor("b2v", [DK, 1], F32, kind="ExternalInput")
    w3aug = nc.dram_tensor("w3aug", [DK + 1, CPC * COUT], F32,
                           kind="ExternalInput")
    y = nc.dram_tensor("y", [COUT, T], F32, kind="ExternalOutput")
    xpad = nc.dram_tensor("xpad", [CPC, XPAD_W], DTC)

    with tile.TileContext(nc) as tc:
        with tc.tile_pool(name="const", bufs=1) as const, \
             tc.tile_pool(name="sb", bufs=1) as sb, \
             tc.tile_pool(name="drn", bufs=2) as drn, \
             tc.tile_pool(name="gt", bufs=1) as gtp, \
             tc.tile_pool(name="xcp", bufs=3) as xcp, \
             tc.tile_pool(name="psg", bufs=4, space="PSUM") as psg, \
             tc.tile_pool(name="psc", bufs=1, space="PSUM") as psc, \
             tc.tile_pool(name="pst", bufs=2, space="PSUM") as pst:

            # ---------- PE prewarm + ACT Sin-table preload ----------
            # ~20 dummy matmuls warm the PE clock gate (HAM) during the DMA/
            # SIREN wait so the matmul stream starts at 2.4 GHz; a dummy Sin
            # loads the ACT table off the h1 critical path.
            warm_src = const.tile([128, 512], DTC, name="warm")
            nc.vector.memset(warm_src[:].bitcast(F32), 0.0)
            pwarm = pst.tile([128, 512], F32, tag="pt")
            for i in range(20):
                nc.tensor.matmul(pwarm[:], warm_src[:, 0:128], warm_src[:],
                                 start=(i == 0), stop=(i == 19),
                                 skip_group_check=True)
            sintab = const.tile([DK, 4], F32, name="sintab")
            nc.scalar.activation(sintab[:], warm_src[0:DK, 0:4], AF.Sin)

            # ---------- SIREN input DMAs first: they head the critical
            # chain (posr -> h1 -> h2 -> Gt2 -> conv) ----------
            posr_t = const.tile([DK, T], F32)
            nc.sync.dma_start(out=posr_t[:], in_=posr.ap())
            w1v_t = const.tile([DK, 1], F32)
            nc.sync.dma_start(out=w1v_t[:], in_=w1v.ap())
            b1v_t = const.tile([DK, 1], F32)
            nc.sync.dma_start(out=b1v_t[:], in_=b1v.ap())
            w2t_t = const.tile([DK, DK], F32)
            nc.sync.dma_start(out=w2t_t[:], in_=w2t.ap())
            b2v_t = const.tile([DK, 1], F32)
            nc.sync.dma_start(out=b2v_t[:], in_=b2v.ap())
            w3aug_t = const.tile([DK + 1, CPC * COUT], F32)
            nc.sync.dma_start(out=w3aug_t[:], in_=w3aug.ap())
            xt = sb.tile([CPC, T], F32)
            nc.sync.dma_start(out=xt[:], in_=xs.ap())

            # h1 = sin(w1v * posr + b1v) in per-chunk ACT ops (per-partition
            # scale), rounded to the conv dtype; chunking lets each h2 matmul
            # start as soon as its quarter of h1 is ready
            h1 = sb.tile([DK, T], DTC)
            for q in range(T // 512):
                nc.scalar.activation(h1[:, q * 512:(q + 1) * 512],
                                     posr_t[:, q * 512:(q + 1) * 512],
                                     AF.Sin, bias=b1v_t[:], scale=w1v_t[:])
            w2r = const.tile([DK, DK], DTC)
            nc.vector.tensor_copy(w2r[:], w2t_t[:])

            # x staging on Vector (fast cast); DMAs on the GpSimd queue so
            # they never queue behind the SIREN-input DMAs on Sync
            zp = sb.tile([CPC, XPAD_W], DTC)
            nc.vector.memset(zp[:, 0:512].bitcast(F32), 0.0)
            nc.vector.tensor_copy(zp[:, 512:XPAD_W], xt[:])
            nc.gpsimd.dma_start(out=xpad.ap(), in_=zp[:])

            # h2r = [sin(om2*(w2 @ h1) + om2*b2); ones], written directly in
            # the conv dtype (whole tile pre-set to 1.0 so row DK is ones)
            h2r = sb.tile([DK + 1, T], DTC)
            if mybir.dt.size(DTC) == 4:
                nc.gpsimd.memset(h2r[:].bitcast(F32), 1.0)  # f32r lacks memset
            else:
                nc.gpsimd.memset(h2r[:], 1.0)
            for q in range(T // 512):
                ph = psg.tile([DK, 512], F32, tag="g")
                nc.tensor.matmul(ph[:], w2r[:], h1[:, q * 512:(q + 1) * 512],
                                 start=True, stop=True)
                nc.scalar.activation(h2r[0:DK, q * 512:(q + 1) * 512], ph[:],
                                     AF.Sin, bias=b2v_t[:], scale=float(om2))

            w3r = sb.tile([DK + 1, CPC * COUT], DTC)
            nc.vector.tensor_copy(w3r[:], w3aug_t[:])

            # short bridge: keeps the PE streaming across the h2->Gt2 handoff
            # (the Sin chain leaves a ~1.5us PE idle window that re-throttles
            # the clock gate for the next ~15us otherwise)
            pwarm2 = pst.tile([128, 512], F32, tag="pt")
            for i in range(6):
                nc.tensor.matmul(pwarm2[:], warm_src[:, 0:128], warm_src[:],
                                 start=(i == 0), stop=(i == 5),
                                 skip_group_check=True)


            # ---------- Gt2, split by input-channel quartet ----------
            # gtq[q][r, jt*256 + (cl%4)*64 + o]; conv for quartet q depends
            # only on gtq[q], so quartet 0 unblocks the conv after 16 copies
            # and the rest of the copies overlap conv matmuls.
            gtq = [gtp.tile([128, 16 * 256], DTC, name=f"gtq{q}")
                   for q in range(4)]
            gtqv = [g[:].rearrange("p (j x) -> p j x", j=16) for g in gtq]

            def emit_gt2_half(half, jts=None):
                for jt in (range(16) if jts is None else jts):
                    pg = psg.tile([128, 512], F32, tag="g")
                    nc.tensor.matmul(
                        pg[:], h2r[:, jt * 128:(jt + 1) * 128],
                        w3r[:, half * 512:(half + 1) * 512],
                        start=True, stop=True)
                    for qh in range(2):
                        q = 2 * half + qh
                        dst = gtq[q][:, jt * 256:(jt + 1) * 256]
                        srcv = pg[:, qh * 256:(qh + 1) * 256]
                        if qh == 0:
                            nc.vector.tensor_copy(dst, srcv)
                        else:
                            nc.scalar.copy(dst, srcv)

            # ---------- causal conv: accumulate in 2 PSUM banks ----------
            # Emission interleaves Gt2 halves with conv channel blocks so the
            # conv starts right after the 16 half-0 Gt2 matmuls.
            psw = [psc.tile([128, 512], F32, name=f"pw{w}") for w in range(2)]

            def emit_conv_cl(cl):
                xc = xcp.tile([128, XC_W], DTC)
                nc.gpsimd.dma_start(
                    out=xc[:],
                    in_=bass.AP(xpad, cl * XPAD_W + 1, [[1, 128], [1, XC_W]]))
                for w in range(2):
                    dmax = 7 if w == 0 else 15
                    for d in range(dmax + 1):
                        beta0 = max(0, d - 8 * w)
                        nb = 8 - beta0
                        j0 = beta0 + 8 * w - d
                        station = xc[:, 128 * d + 384: 128 * d + 512]
                        q, clq = divmod(cl, 4)
                        moving = gtqv[q][:, j0:j0 + nb, clq * 64:(clq + 1) * 64]
                        nc.tensor.matmul(
                            psw[w][:, beta0 * 64: 512], station, moving,
                            start=(cl == 0 and d == 0),
                            stop=(cl == CPC - 1 and d == dmax),
                            skip_group_check=True)

            emit_gt2_half(0)
            for cl in range(0, 4):
                emit_conv_cl(cl)
            for cl in range(4, 8):
                # spread the half-1 Gt2 matmuls between conv blocks to keep
                # the PE duty cycle high (a contiguous block re-throttles HAM)
                emit_gt2_half(1, jts=range(4 * (cl - 4), 4 * (cl - 3)))
                emit_conv_cl(cl)
            for cl in range(8, CPC):
                emit_conv_cl(cl)

            # ---------- epilogue: transpose (tloc, (beta,o)) -> (o, t) ----------
            # f32r operands: single-pass transpose at 1.5 cyc/row (vs 4 for
            # fp32) with ~1e-4 rounding, far below the conv dtype error
            F32R = mybir.dt.float32r
            identf = const.tile([128, 128], F32)
            make_identity(nc, identf[:])
            ident = const.tile([128, 128], F32R)
            nc.vector.tensor_copy(ident[:], identf[:])
            for w in range(2):
                out_sb = drn.tile([COUT, T // 2], F32, name=f"osb{w}", bufs=1)
                sb_d = drn.tile([128, 512], F32R)
                nc.vector.tensor_copy(sb_d[:], psw[w][:])
                for beta in range(8):
                    pt = pst.tile([COUT, 128], F32R)
                    nc.tensor.transpose(pt[:], sb_d[:, beta * 64:(beta + 1) * 64],
                                        ident[:])
                    dst = out_sb[:, beta * 128:(beta + 1) * 128]
                    if beta % 2 == 0:
                        nc.vector.tensor_copy(dst, pt[:])
                    else:
                        nc.scalar.copy(dst, pt[:])
                yv = y.ap().rearrange("o (w t) -> o w t", w=2)[:, w, :]
                nc.sync.dma_start(out=yv, in_=out_sb[:])

    nc.compile()
    return nc


def kernel(x, pos_rel, w1, b1, om1, w2, b2, om2, w3, b3, bias,
           dt_conv_name: str = "bfloat16", _trace_tmpdir=None):
    from concourse.bass_utils import run_bass_kernel_spmd

    x = np.asarray(x, dtype=np.float32)
    pos_rel = np.asarray(pos_rel, dtype=np.float32)
    w1 = np.asarray(w1, dtype=np.float32)
    b1 = np.asarray(b1, dtype=np.float32)
    om1 = float(np.asarray(om1))
    w2 = np.asarray(w2, dtype=np.float32)
    b2 = np.asarray(b2, dtype=np.float32)
    om2 = float(np.asarray(om2))
    w3 = np.asarray(w3, dtype=np.float32)
    b3 = np.asarray(b3, dtype=np.float32)
    bias = np.asarray(bias, dtype=np.float32)

    # block-reversed positions (within each 128-tap tile), taps 0..2047 only,
    # replicated to DK partitions for the broadcast-free h1 compute
    posr_row = pos_rel[:T].reshape(T // 128, 128)[:, ::-1].reshape(T)
    posr = np.ascontiguousarray(
        np.broadcast_to(posr_row[None, :], (DK, T)), dtype=np.float32)

    w1v = (om1 * w1).reshape(DK, 1).astype(np.float32)
    b1v = (om1 * b1).reshape(DK, 1).astype(np.float32)
    w2t = np.ascontiguousarray(w2.T, dtype=np.float32)
    b2v = b2.reshape(DK, 1).astype(np.float32)  # om2 applied as ACT scale

    nc = _build_program(om2, dt_conv_name)

    # per-core inputs
    in_maps = []
    for core in range(N_CORES):
        b, h = divmod(core, 2)
        ci0 = h * CPC
        # w3aug[d, cl*64 + o] = w3[o*32 + ci0 + cl, d]; row DK = b3 slice
        w3_r = w3.reshape(COUT, CIN, DK)[:, ci0:ci0 + CPC, :]   # (o, cl, d)
        w3a = np.transpose(w3_r, (2, 1, 0)).reshape(DK, CPC * COUT)  # d,(cl,o)
        b3_r = b3.reshape(COUT, CIN)[:, ci0:ci0 + CPC]          # (o, cl)
        b3a = np.transpose(b3_r, (1, 0)).reshape(1, CPC * COUT)  # (cl, o)
        w3aug = np.concatenate([w3a, b3a], axis=0).astype(np.float32)
        in_maps.append({
            "xs": np.ascontiguousarray(x[b, ci0:ci0 + CPC, :]),
            "posr": posr,
            "w1v": w1v, "b1v": b1v, "w2t": w2t, "b2v": b2v,
            "w3aug": np.ascontiguousarray(w3aug),
        })

    kwargs = {}
    if _trace_tmpdir is not None:
        kwargs = dict(trace=True, tmpdir=_trace_tmpdir)
    res = run_bass_kernel_spmd(nc, in_maps, list(range(N_CORES)), **kwargs)

    out = np.empty((B, COUT, T), dtype=np.float32)
    for b in range(B):
        out[b] = res.results[2 * b]["y"] + res.results[2 * b + 1]["y"]
    out += bias[None, :, None]
    if _trace_tmpdir is not None:
        kernel.last_exec_time_ns = res.exec_time_ns
    return out



# revision 8
# speedup vs baseline: 1.1483x; 1.1483x over previous
"""CKConv (continuous-kernel causal conv) Trainium2 Bass kernel.

Problem: out[b,o,t] = sum_{ci,k<=t} g[o,ci,k] * x[b,ci,t-k] + bias[o]
with g generated by a tiny SIREN net on relative positions.
Shapes: B=4, CIN=32, COUT=64, T=2048, kernel length K=T+1 (tap 2048 never
contributes for t < T, so only taps 0..2047 are computed).

Sharding: 8 cores = (batch b in 0..3) x (input-channel half h in 0..1).
Each core computes a partial over its 16 input channels for all 64 output
channels; the host adds the two halves and the bias (exact fp32 adds).

Conv formulation (x-stationary): time tiles of 128. For output tile tt and
tap tile j, the contribution is Xwin(d=tt-j).T @ G(j) where Xwin(d)[r, tloc]
= xpad(128d + tloc + r - 127) is a 128x128 window of the shifted-replicated
input (im2col by a single overlapping-window DMA from the host-prepadded
bf16 input, partition step +1), and G(j)[r, o] = g[o, cl, 128j + 127 - r].
The within-tile tap reversal comes free from a block-reversed position
vector fed to the SIREN.  One matmul per (cl, group, d) covers all valid
beta blocks at once (moving operand with 2 free dims).

Output accumulates in 3 PSUM groups -- A: t in [0,1024), B: [1024,1536),
C: [1536,2048) -- emitted cl-major so A/B stop early and their drains
(PSUM->SBUF cast, PE transposes, copies, DMA out) hide under the tail of
the C conv stream; only C's small drain is exposed.

SIREN is packed across partitions to kill the head latency: h1 as
[64, 512] (4 position blocks x 16 chans), h2 via a block-diagonal [64,128]
stationary into [128, 512] where each 32-partition block holds 16 d2 rows
+ a ones row (ACT Sin with bias pi/2 on a zero input), and Gt2 contracts
all 128 partitions against a 4x-replicated zero-padded w3 so the padding
rows vanish.  HAM (PE clock gate) is kept warm with a short warmup burst
plus 128-col filler matmuls through the drain-rate-bound Gt2 phase.

Matmul dtype bfloat16: ~3e-3 max-rel / ~3e-4 rms-rel error.
"""

import numpy as np

B, CIN, COUT, T = 4, 32, 64, 2048
DK = 16
N_CORES = 8
CPC = CIN // 2          # channels per core = 16
XPAD_W = 2560           # 512 left zeros + 2048 data (host pre-padded)
XC_W = 2432             # im2col window columns
NW1 = 6                 # HAM warmup matmuls before the h2 matmul
NW2 = 3                 # bridge matmuls covering the h2 Sin window


def _build_program(om2: float, dt_conv_name: str):
    import concourse.bass as bass
    import concourse.mybir as mybir
    import concourse.tile as tile
    from concourse import bacc
    from concourse.masks import make_identity

    F32 = mybir.dt.float32
    F32R = mybir.dt.float32r
    DTC = getattr(mybir.dt, dt_conv_name)
    AF = mybir.ActivationFunctionType

    nc = bacc.Bacc("TRN2", target_bir_lowering=False, debug=False,
                   num_devices=N_CORES)

    xsp = nc.dram_tensor("xsp", [CPC, XPAD_W], DTC, kind="ExternalInput")
    pf32 = nc.dram_tensor("pf32", [128, 515], F32, kind="ExternalInput")
    pbf = nc.dram_tensor("pbf", [128, 4224], DTC, kind="ExternalInput")
    y = nc.dram_tensor("y", [COUT, T], F32, kind="ExternalOutput")

    with tile.TileContext(nc) as tc:
        with tc.tile_pool(name="const", bufs=1) as const, \
             tc.tile_pool(name="sb", bufs=1) as sb, \
             tc.tile_pool(name="sbd", bufs=3) as sbd, \
             tc.tile_pool(name="outp", bufs=3) as outp, \
             tc.tile_pool(name="gt", bufs=1) as gtp, \
             tc.tile_pool(name="xcp", bufs=3) as xcp, \
             tc.tile_pool(name="psg", bufs=4, space="PSUM") as psg, \
             tc.tile_pool(name="psc", bufs=1, space="PSUM") as psc, \
             tc.tile_pool(name="pst", bufs=2, space="PSUM") as pst:

            # ---------- head: warm source + ACT Sin-table preload ----------
            warm = const.tile([128, 512], DTC, name="warm")
            nc.vector.memset(warm[:].bitcast(F32), 0.0)
            sintab = const.tile([DK, 4], F32, name="sintab")
            nc.scalar.activation(sintab[:], warm[0:DK, 0:4], AF.Sin)

            # ---------- param DMAs (sync queue) ----------
            pf32_t = const.tile([128, 515], F32)
            nc.sync.dma_start(out=pf32_t[:], in_=pf32.ap())
            pbf_t = const.tile([128, 4224], DTC)
            nc.sync.dma_start(out=pbf_t[:, 0:2176], in_=pbf.ap()[:, 0:2176])
            nc.sync.dma_start(out=pbf_t[:, 2176:4224],
                              in_=pbf.ap()[:, 2176:4224])

            b2v2 = pf32_t[:, 0:1]
            posr2 = pf32_t[0:64, 1:513]
            w1v2 = pf32_t[0:64, 513:514]
            b1v2 = pf32_t[0:64, 514:515]
            w2big = pbf_t[0:64, 0:128]
            w3sel = pbf_t[:, 128:4224]      # [128, 4096]: 4 tb blocks x 1024

            # ---------- first im2col window DMAs (sync queue) ----------
            xcts = {}

            def ensure_xc(cl):
                if cl in xcts or cl >= CPC:
                    return
                t = xcp.tile([128, XC_W], DTC)
                nc.sync.dma_start(
                    out=t[:],
                    in_=bass.AP(xsp, cl * XPAD_W + 1, [[1, 128], [1, XC_W]]))
                xcts[cl] = t

            for cl in range(3):
                ensure_xc(cl)

            # ---------- HAM warmup burst (cold ~427ns each) ----------
            pwarm = psg.tile([128, 512], F32, tag="g")
            for i in range(NW1):
                nc.tensor.matmul(pwarm[:], warm[:, 0:128], warm[:],
                                 start=(i == 0), stop=(i == NW1 - 1),
                                 skip_group_check=True)

            # ---------- SIREN, partition-packed ----------
            # h1[(tb,d1), tl] = sin(om1*w1[d1]*pos[512*tb+tl] + om1*b1[d1])
            h1b = sb.tile([64, 512], DTC)
            nc.scalar.activation(h1b[:], posr2, AF.Sin,
                                 bias=b1v2, scale=w1v2)
            # h2p[(tb,d2'), tl] = sum_d1 w2[d2',d1] h1[(tb,d1), tl]
            # (block-diagonal stationary; d2'=16 ones-row and pad rows get 0)
            h2p = psg.tile([128, 512], F32, tag="g")
            nc.tensor.matmul(h2p[:], w2big, h1b[:], start=True, stop=True)
            # bridge the h2 Sin window so the PE stream never dips
            pwarm2 = psg.tile([128, 512], F32, tag="g")
            for i in range(NW2):
                nc.tensor.matmul(pwarm2[:], warm[:, 0:128], warm[:],
                                 start=(i == 0), stop=(i == NW2 - 1),
                                 skip_group_check=True)
            # h2r = sin(om2*h2p + om2*b2); ones rows get sin(pi/2) = 1
            h2r = sb.tile([128, 512], DTC)
            nc.scalar.activation(h2r[:], h2p[:], AF.Sin,
                                 bias=b2v2, scale=float(om2))

            # ---------- Gt2: g[o,cl,128j+127-r] -> gtq[cl//4][r, j, (cl%4)*64+o]
            gtq = [gtp.tile([128, 16 * 256], DTC, name=f"gtq{q}")
                   for q in range(4)]
            gtqv = [g[:].rearrange("p (j x) -> p j x", j=16) for g in gtq]

            drain_eng = [0]

            def emit_drains(pg, half, jt):
                # GPSIMD cannot read PSUM; alternate the two capable engines
                # across quartets per jt so neither becomes the fixed laggard
                for qh in range(2):
                    q = 2 * half + qh
                    src = pg[:, qh * 256:(qh + 1) * 256]
                    dst = gtq[q][:, jt * 256:(jt + 1) * 256]
                    if (qh + jt) % 2 == 0:
                        nc.vector.tensor_copy(dst, src)
                    else:
                        nc.scalar.copy(dst, src)

            def emit_gt2(half, jts, fillers=False):
                for jt in jts:
                    pg = psg.tile([128, 512], F32, tag="g")
                    rhs = w3sel[:, (jt // 4) * 1024 + half * 512:
                                (jt // 4) * 1024 + half * 512 + 512]
                    nc.tensor.matmul(
                        pg[:], h2r[:, (jt % 4) * 128:(jt % 4) * 128 + 128],
                        rhs, start=True, stop=True)
                    if fillers:
                        # 256-col filler keeps HAM window activity high while
                        # the Gt2 stream runs drain-rate-bound; pA is safe
                        # scratch here -- conv A's start=True reset comes later
                        nc.tensor.matmul(pA[:, 0:256], warm[:, 0:128],
                                         warm[:, 0:256], start=True,
                                         stop=True, skip_group_check=True)
                    emit_drains(pg, half, jt)

            # ---------- conv: 3 PSUM groups, cl-major ----------
            pA = psc.tile([128, 512], F32, name="pA")
            pBC = psc.tile([128, 512], F32, name="pBC")
            # B and C share this bank; a matmul start=True would reset the
            # whole bank and wipe the other group's partials -- zero it once
            # and accumulate with start=False throughout
            nc.vector.memset(pBC[:], 0.0)

            def emit_conv(cl, grp, dlist=None):
                xc = xcts[cl]
                q, clq = divmod(cl, 4)
                if grp == 'A':          # w=0: tt = beta, t in [0, 1024)
                    for d in (dlist or range(8)):
                        beta0 = d
                        nb = 8 - beta0
                        station = xc[:, 128 * d + 384: 128 * d + 512]
                        moving = gtqv[q][:, 0:nb, clq * 64:(clq + 1) * 64]
                        nc.tensor.matmul(
                            pA[:, beta0 * 64: 512], station, moving,
                            start=(cl == 0 and d == 0),
                            stop=(cl == CPC - 1 and d == 7),
                            skip_group_check=True)
                elif grp == 'B':        # w=1 beta 0..3: t in [1024, 1536)
                    for d in (dlist or range(12)):
                        beta0 = max(0, d - 8)
                        nb = 4 - beta0
                        j0 = 8 + beta0 - d
                        station = xc[:, 128 * d + 384: 128 * d + 512]
                        moving = gtqv[q][:, j0:j0 + nb,
                                         clq * 64:(clq + 1) * 64]
                        nc.tensor.matmul(
                            pBC[:, beta0 * 64: 256], station, moving,
                            start=False,
                            stop=(cl == CPC - 1 and d == 11),
                            skip_group_check=True)
                else:                   # 'C' w=1 beta 4..7: t in [1536, 2048)
                    for d in (dlist if dlist is not None else range(16)):
                        beta0 = max(0, d - 12)
                        nb = 4 - beta0
                        j0 = 12 + beta0 - d
                        station = xc[:, 128 * d + 384: 128 * d + 512]
                        moving = gtqv[q][:, j0:j0 + nb,
                                         clq * 64:(clq + 1) * 64]
                        nc.tensor.matmul(
                            pBC[:, 256 + beta0 * 64: 512], station, moving,
                            start=False,
                            stop=(cl == CPC - 1 and d == 15),
                            skip_group_check=True)

            # Gt2 half 0 (quartets 0,1 = cl 0..7), drains filler-padded
            emit_gt2(0, range(16), fillers=True)

            # conv cl 0..3; then cl 4..7 with Gt2 half 1 spread in bursts
            for cl in range(0, 4):
                ensure_xc(cl + 2)
                emit_conv(cl, 'A')
                emit_conv(cl, 'B')
                emit_conv(cl, 'C')
            for cl in range(4, 8):
                ensure_xc(cl + 2)
                j0 = (cl - 4) * 4
                emit_gt2(1, range(j0, j0 + 2))
                emit_conv(cl, 'A')
                emit_gt2(1, range(j0 + 2, j0 + 3))
                emit_conv(cl, 'B')
                emit_gt2(1, range(j0 + 3, j0 + 4))
                emit_conv(cl, 'C')
            for cl in range(8, CPC - 1):
                ensure_xc(cl + 2)
                if cl == 10:
                    # transpose identity prep on idle engines, mid-conv
                    identf = const.tile([128, 128], F32, name="identf")
                    make_identity(nc, identf[:])
                    ident = const.tile([128, 128], F32R, name="ident")
                    nc.vector.tensor_copy(ident[:], identf[:])
                emit_conv(cl, 'A')
                emit_conv(cl, 'B')
                emit_conv(cl, 'C')

            # ---------- cl 15 + interleaved drains ----------
            cl = CPC - 1
            emit_conv(cl, 'A')
            emit_conv(cl, 'B')

            # A drain: cast runs under B(15); transposes slot in before C
            sb_dA = sbd.tile([128, 512], F32R, name="sbdA")
            nc.vector.tensor_copy(sb_dA[:], pA[:])
            outA = outp.tile([COUT, 1024], F32, name="outA")
            for b8 in range(8):
                pt = pst.tile([COUT, 128], F32R)
                nc.tensor.transpose(pt[:], sb_dA[:, b8 * 64:(b8 + 1) * 64],
                                    ident[:])
                dst = outA[:, b8 * 128:(b8 + 1) * 128]
                if b8 % 2 == 0:
                    nc.vector.tensor_copy(dst, pt[:])
                else:
                    nc.scalar.copy(dst, pt[:])

            emit_conv(cl, 'C', dlist=range(0, 8))
            nc.sync.dma_start(out=y.ap()[:, 0:1024], in_=outA[:])

            # B drain under C(15) tail
            sb_dB = sbd.tile([128, 256], F32R, name="sbdB")
            nc.vector.tensor_copy(sb_dB[:], pBC[:, 0:256])
            outB = outp.tile([COUT, 512], F32, name="outB")
            for b4 in range(4):
                pt = pst.tile([COUT, 128], F32R)
                nc.tensor.transpose(pt[:], sb_dB[:, b4 * 64:(b4 + 1) * 64],
                                    ident[:])
                dst = outB[:, b4 * 128:(b4 + 1) * 128]
                if b4 % 2 == 0:
                    nc.vector.tensor_copy(dst, pt[:])
                else:
                    nc.scalar.copy(dst, pt[:])

            emit_conv(cl, 'C', dlist=range(8, 16))
            nc.sync.dma_start(out=y.ap()[:, 1024:1536], in_=outB[:])

            # C drain: the only exposed tail
            sb_dC = sbd.tile([128, 256], F32R, name="sbdC")
            nc.vector.tensor_copy(sb_dC[:], pBC[:, 256:512])
            outC = outp.tile([COUT, 512], F32, name="outC")
            for b4 in range(4):
                pt = pst.tile([COUT, 128], F32R)
                nc.tensor.transpose(pt[:], sb_dC[:, b4 * 64:(b4 + 1) * 64],
                                    ident[:])
                dst = outC[:, b4 * 128:(b4 + 1) * 128]
                if b4 % 2 == 0:
                    nc.vector.tensor_copy(dst, pt[:])
                else:
                    nc.scalar.copy(dst, pt[:])
            nc.sync.dma_start(out=y.ap()[:, 1536:2048], in_=outC[:])

    nc.compile()
    return nc


def kernel(x, pos_rel, w1, b1, om1, w2, b2, om2, w3, b3, bias,
           dt_conv_name: str = "bfloat16", _trace_tmpdir=None):
    import ml_dtypes
    from concourse.bass_utils import run_bass_kernel_spmd

    x = np.asarray(x, dtype=np.float32)
    pos_rel = np.asarray(pos_rel, dtype=np.float32)
    w1 = np.asarray(w1, dtype=np.float32)
    b1 = np.asarray(b1, dtype=np.float32)
    om1 = float(np.asarray(om1))
    w2 = np.asarray(w2, dtype=np.float32)
    b2 = np.asarray(b2, dtype=np.float32)
    om2 = float(np.asarray(om2))
    w3 = np.asarray(w3, dtype=np.float32)
    b3 = np.asarray(b3, dtype=np.float32)
    bias = np.asarray(bias, dtype=np.float32)
    bf16 = ml_dtypes.bfloat16

    # block-reversed positions (within each 128-tap tile), taps 0..2047
    posrow = pos_rel[:T].reshape(T // 128, 128)[:, ::-1].reshape(T)

    # pf32 [128, 515]: col 0 = b2 bias per (tb,d2') 32-block (pi/2 on the
    # ones/pad rows); cols 1:513 = positions per (tb,d1); 513/514 = om1*w1/b1
    pf32 = np.zeros((128, 515), np.float32)
    pf32[:, 0] = np.pi / 2
    for tb in range(4):
        pf32[32 * tb:32 * tb + 16, 0] = om2 * b2
        pf32[16 * tb:16 * tb + 16, 1:513] = posrow[512 * tb:512 * (tb + 1)]
        pf32[16 * tb:16 * tb + 16, 513] = om1 * w1.reshape(DK)
        pf32[16 * tb:16 * tb + 16, 514] = om1 * b1

    # W2big [64, 128]: block-diagonal w2.T; cols (tb,16..31) zero
    w2big = np.zeros((64, 128), np.float32)
    for tb in range(4):
        w2big[16 * tb:16 * tb + 16, 32 * tb:32 * tb + 16] = w2.T

    nc = _build_program(om2, dt_conv_name)

    in_maps = []
    for core in range(N_CORES):
        b, h = divmod(core, 2)
        ci0 = h * CPC
        # w3a[d, cl*64 + o] = w3[o*CIN + ci0 + cl, d]; b3a = matching b3 row
        w3_r = w3.reshape(COUT, CIN, DK)[:, ci0:ci0 + CPC, :]
        w3a = np.transpose(w3_r, (2, 1, 0)).reshape(DK, CPC * COUT)
        b3_r = b3.reshape(COUT, CIN)[:, ci0:ci0 + CPC]
        b3a = np.transpose(b3_r, (1, 0)).reshape(CPC * COUT)

        # pbf [128, 4224]: cols 0:128 = W2big (rows 0:64); 128: = w3sel,
        # 4 tb-blocks of 1024 cols with 17 live rows at 32-partition stride
        pbf = np.zeros((128, 4224), np.float32)
        pbf[0:64, 0:128] = w2big
        for tb in range(4):
            c0 = 128 + 1024 * tb
            pbf[32 * tb:32 * tb + 16, c0:c0 + 1024] = w3a
            pbf[32 * tb + 16, c0:c0 + 1024] = b3a

        xsp = np.zeros((CPC, XPAD_W), np.float32)
        xsp[:, 512:] = x[b, ci0:ci0 + CPC, :]

        in_maps.append({
            "xsp": xsp.astype(bf16),
            "pf32": pf32,
            "pbf": pbf.astype(bf16),
        })

    kwargs = {}
    if _trace_tmpdir is not None:
        kwargs = dict(trace=True, tmpdir=_trace_tmpdir)
    res = run_bass_kernel_spmd(nc, in_maps, list(range(N_CORES)), **kwargs)

    out = np.empty((B, COUT, T), dtype=np.float32)
    for b in range(B):
        out[b] = res.results[2 * b]["y"] + res.results[2 * b + 1]["y"]
    out += bias[None, :, None]
    if _trace_tmpdir is not None:
        kernel.last_exec_time_ns = res.exec_time_ns
    return out
